# revision 1
# baseline (speedup 1.0000x reference)
"""AttentiveDensenet Trainium2 Bass kernel.

Data-parallel over batch B=8 across 8 NeuronCores (1 image per core).
Per layer l (of 4):
  - K/Q/V 1x1 convs as bf16 matmuls with x-tiles as the stationary operand,
    producing position-major [pos, (head, dim)] activations directly
    (avoids any transpose for the attention stage). Bias via a K=1
    ones-row matmul accumulated into PSUM.
  - Attention is per-token over the growing key/val list: score products on
    DVE (bf16, 2x mode), d-reduction on DVE, softmax + exact top-k
    (2nd-smallest via min-of-pairwise-max) on DVE/ACT, weighted sum on
    GPSIMD (products) + DVE (accumulate).
  - o is cast to bf16, bounced through DRAM, and transposed to channel-major
    padded layout with the DMA xbar transpose engine.
  - conv3x3 #1 as 9 shifted 1x1 convs accumulated in PSUM (bf16).
  - BatchNorm stats: per-core per-channel sum/sumsq, AllGathered across the
    8 cores (2KB), summed locally. Training-mode BN; the conv bias ob1
    cancels exactly in BN and is dropped.
  - h1 = relu(A*y1 + B) fused on the scalar engine, written bf16 into the
    padded conv2 input.
  - conv3x3 #2 (bf16) + residual x += gamma*(h2 + ob2) fused via
    scalar_tensor_tensor from PSUM.
"""
import numpy as np
import ml_dtypes

import concourse.bacc as bacc
import concourse.mybir as mybir
import concourse.tile as tile
from concourse import bass_utils

L, C, B, H, W = 4, 256, 8, 32, 32
NH, KD = 8, 64
KH = NH * KD          # 512
HW = H * W            # 1024
P = 128
NC = 8                # cores
TOPK = 4
EPS = 1e-7
BN_EPS = 1e-5
PW = W + 2            # 34
PHW = PW * (H + 2)    # 1156

f32 = mybir.dt.float32
bf16 = mybir.dt.bfloat16
AX = mybir.AxisListType
OP = mybir.AluOpType
ACTF = mybir.ActivationFunctionType

_compiled = {}
DBGL = 0


def _build(ncores=NC, dbg=False, no_cc=False, no_gps=False, no_xpose=False, dense_rhs=False, layers=L, stages=99):
    nc = bacc.Bacc(None, target_bir_lowering=False, debug=False, num_devices=ncores)

    # ---- DRAM I/O (per-core shapes) ----
    xin = nc.dram_tensor("xin", [C, HW], f32, kind="ExternalInput").ap()
    wq = nc.dram_tensor("wq", [L, 2, P, KH], bf16, kind="ExternalInput").ap()
    wk = nc.dram_tensor("wk", [L, 2, P, KH], bf16, kind="ExternalInput").ap()
    wv = nc.dram_tensor("wv", [L, 2, P, KH], bf16, kind="ExternalInput").ap()
    bq = nc.dram_tensor("bq", [L, 1, KH], bf16, kind="ExternalInput").ap()
    bk = nc.dram_tensor("bk", [L, 1, KH], bf16, kind="ExternalInput").ap()
    bv = nc.dram_tensor("bv", [L, 1, KH], bf16, kind="ExternalInput").ap()
    w1 = nc.dram_tensor("w1", [L, 9, 4, 2, P, P], bf16, kind="ExternalInput").ap()
    w2 = nc.dram_tensor("w2", [L, 9, 2, 2, P, P], bf16, kind="ExternalInput").ap()
    bngd = nc.dram_tensor("bngd", [L, 2, P, 1], f32, kind="ExternalInput").ap()
    bnbd = nc.dram_tensor("bnbd", [L, 2, P, 1], f32, kind="ExternalInput").ap()
    gob2d = nc.dram_tensor("gob2d", [L, 2, P, 1], f32, kind="ExternalInput").ap()
    gamd = nc.dram_tensor("gamd", [L, P, 1], f32, kind="ExternalInput").ap()
    out = nc.dram_tensor("out", [C, HW], f32, kind="ExternalOutput").ap()
    dbgt = {}
    if dbg:
        for nm, shp in [("d_q", [P, 8 * KH]), ("d_k", [P, 8 * KH]), ("d_v", [P, 8 * KH]),
                        ("d_S", [P, 320]), ("d_attn", [P, 320]), ("d_o", [P, 8 * KH]),
                        ("d_opad0", [P, PHW]), ("d_y1_0", [P, HW]), ("d_gsum", [P, 4]),
                        ("d_A0", [P, 1]), ("d_B0", [P, 1]), ("d_h1p0", [P, PHW]),
                        ("d_x0", [P, HW])]:
            dbgt[nm] = nc.dram_tensor(nm, shp, f32, kind="ExternalOutput").ap()

    with tile.TileContext(nc) as tc:
        with tc.tile_pool(name="main", bufs=1) as mp, \
             tc.tile_pool(name="prodp", bufs=2) as prodp, \
             tc.tile_pool(name="tmpp", bufs=2) as tmpp, \
             tc.tile_pool(name="wkvp", bufs=4) as wkvp, \
             tc.tile_pool(name="wcp", bufs=12) as wcp, \
             tc.tile_pool(name="biasp", bufs=3) as biasp, \
             tc.tile_pool(name="kqvps", bufs=4, space="PSUM") as kqvps, \
             tc.tile_pool(name="convps", bufs=4, space="PSUM") as convps, \
             tc.tile_pool(name="dramp", bufs=2, space="DRAM") as dramp:

            # persistent tiles
            x = [mp.tile([P, HW], f32, name=f"x{i}") for i in range(2)]
            xb = [mp.tile([P, HW], bf16, name=f"xb{i}") for i in range(2)]
            qbt = mp.tile([P, 8 * KH], bf16, name="qbt")
            kbt = [mp.tile([P, 8 * KH], bf16, name=f"kbt{i}") for i in range(L)]
            vbt = [mp.tile([P, 8 * KH], bf16, name=f"vbt{i}") for i in range(L)]
            S = mp.tile([P, 64 * 5], f32, name="S")
            attn = mp.tile([P, 64 * 5], f32, name="attn")
            attnb = mp.tile([P, 64 * 5], bf16, name="attnb")
            mx = mp.tile([P, 64], f32, name="mx")
            zs = mp.tile([P, 64], f32, name="zs")
            dmin = mp.tile([P, 64], f32, name="dmin")
            mxp = mp.tile([P, 64], f32, name="mxp")
            o = mp.tile([P, 8 * KH], f32, name="o")
            obf = mp.tile([P, 8 * KH], bf16, name="obf")
            opad = [mp.tile([P, PHW + 2], bf16, name=f"opad{i}") for i in range(4)]
            y1 = [mp.tile([P, HW], f32, name=f"y1_{i}") for i in range(2)]
            h1p = [mp.tile([P, PHW + 2], bf16, name=f"h1p{i}") for i in range(2)]
            st = mp.tile([P, 4], f32, name="st")
            gst = mp.tile([P, 32], f32, name="gst")
            gsum = mp.tile([P, 4], f32, name="gsum")
            ones1 = mp.tile([1, P], bf16, name="ones1")
            # per-layer consts (reloaded each layer)
            bngt = [mp.tile([P, 1], f32, name=f"bngt{i}") for i in range(2)]
            bnbt = [mp.tile([P, 1], f32, name=f"bnbt{i}") for i in range(2)]
            gob2t = [mp.tile([P, 1], f32, name=f"gob2t{i}") for i in range(2)]
            gamt = mp.tile([P, 1], f32, name="gamt")
            # BN scratch
            t1 = [mp.tile([P, 1], f32, name=f"t1_{i}") for i in range(2)]
            Ac = [mp.tile([P, 1], f32, name=f"Ac{i}") for i in range(2)]
            Bc = [mp.tile([P, 1], f32, name=f"Bc{i}") for i in range(2)]
            sq = mp.tile([P, 1], f32, name="sq")
            vart = mp.tile([P, 1], f32, name="vart")
            stdt = mp.tile([P, 1], f32, name="stdt")

            # init
            for i in range(2):
                nc.sync.dma_start(x[i][:], xin[i * P:(i + 1) * P, :])
                nc.scalar.copy(xb[i][:], x[i][:])
            for i in range(4):
                nc.vector.memset(opad[i][:], 0)
            for i in range(2):
                nc.vector.memset(h1p[i][:], 0)
            nc.vector.memset(ones1[:], 1.0)
            nc.vector.memset(S[:], 0)
            nc.vector.memset(attn[:], 0)

            S3 = S[:].rearrange("p (g t) -> p g t", t=5)
            at3 = attn[:].rearrange("p (g t) -> p g t", t=5)
            ab3 = attnb[:].rearrange("p (g t) -> p g t", t=5)

            for l in range(layers):
                R = l + 1      # number of real keys
                T = R + 1      # +1 zero key

                # ---- per-layer consts ----
                for i in range(2):
                    nc.sync.dma_start(bngt[i][:], bngd[l, i])
                    nc.sync.dma_start(bnbt[i][:], bnbd[l, i])
                    nc.sync.dma_start(gob2t[i][:], gob2d[l, i])
                nc.sync.dma_start(gamt[:], gamd[l])

                # ---- K/Q/V 1x1 convs, position-major ----
                for name, wdr, bdr, dest in (
                    ("k", wk, bk, kbt[l][:]),
                    ("v", wv, bv, vbt[l][:]),
                    ("q", wq, bq, qbt[:]),
                ):
                    bt = biasp.tile([1, KH], bf16, name=f"bias_{name}_{l}", tag="bias")
                    nc.sync.dma_start(bt[:], bdr[l])
                    wts = []
                    for ct in range(2):
                        wt = wkvp.tile([P, KH], bf16, name=f"w_{name}_{l}_{ct}", tag="wkv")
                        nc.sync.dma_start(wt[:], wdr[l, ct])
                        wts.append(wt)
                    for pb in range(8):
                        ps = kqvps.tile([P, KH], f32, name="kqv_ps")
                        nc.tensor.matmul(ps[:], ones1[:], bt[:], start=True, stop=False)
                        nc.tensor.matmul(ps[:], xb[0][:, pb * P:(pb + 1) * P], wts[0][:],
                                         start=False, stop=False)
                        nc.tensor.matmul(ps[:], xb[1][:, pb * P:(pb + 1) * P], wts[1][:],
                                         start=False, stop=True)
                        nc.scalar.copy(dest[:, pb * KH:(pb + 1) * KH], ps[:])

                # ---- scores ----
                if stages < 2: continue
                for t in range(R):
                    pr = prodp.tile([P, 8 * KH], bf16, name="prodb")
                    nc.vector.tensor_mul(pr[:], qbt[:], kbt[t][:])
                    nc.vector.tensor_reduce(
                        out=S3[:, :, t], in_=pr[:].rearrange("p (g d) -> p g d", d=KD),
                        axis=AX.X, op=OP.add)
                nc.vector.memset(S3[:, :, R:R + 1], 0)  # zero key

                # ---- softmax over T slots ----
                if stages < 3: continue
                nc.vector.tensor_reduce(out=mx[:], in_=S3[:, :, 0:T], axis=AX.X, op=OP.max)
                nc.vector.tensor_tensor(
                    at3[:, :, 0:T], S3[:, :, 0:T],
                    mx[:].unsqueeze(2).broadcast_to([P, 64, T]), OP.subtract)
                nc.scalar.activation(at3[:, :, 0:T], at3[:, :, 0:T], ACTF.Exp)
                nc.vector.tensor_reduce(out=zs[:], in_=at3[:, :, 0:T], axis=AX.X, op=OP.add)
                nc.vector.reciprocal(zs[:], zs[:])
                nc.vector.tensor_tensor(
                    at3[:, :, 0:T], at3[:, :, 0:T],
                    zs[:].unsqueeze(2).broadcast_to([P, 64, T]), OP.mult)

                # ---- sparse top-k (only T=5) ----
                if T > TOPK:
                    first = True
                    for i in range(T):
                        for j in range(i + 1, T):
                            dst = dmin if first else mxp
                            nc.vector.tensor_tensor(
                                dst[:], at3[:, :, i], at3[:, :, j],
                                OP.max)
                            if not first:
                                nc.vector.tensor_tensor(dmin[:], dmin[:], mxp[:], OP.min)
                            first = False
                    nc.vector.tensor_scalar_add(dmin[:], dmin[:], EPS)
                    nc.vector.tensor_tensor(
                        at3[:, :, 0:T], at3[:, :, 0:T],
                        dmin[:].unsqueeze(2).broadcast_to([P, 64, T]), OP.subtract)
                    nc.vector.tensor_scalar_max(at3[:, :, 0:T], at3[:, :, 0:T], 0.0)
                    nc.vector.tensor_reduce(out=zs[:], in_=at3[:, :, 0:T], axis=AX.X,
                                            op=OP.add)
                    nc.vector.tensor_scalar_add(zs[:], zs[:], EPS)
                    nc.vector.reciprocal(zs[:], zs[:])
                    nc.vector.tensor_tensor(
                        at3[:, :, 0:T], at3[:, :, 0:T],
                        zs[:].unsqueeze(2).broadcast_to([P, 64, T]), OP.mult)

                nc.vector.tensor_copy(attnb[:], attn[:])

                # ---- weighted sum: o = sum_t attn_t * v_t ----
                if stages < 4: continue
                o3 = o[:].rearrange("p (g d) -> p g d", d=KD)
                for t in range(R):
                    v3 = vbt[t][:].rearrange("p (g d) -> p g d", d=KD)
                    ab = ab3[:, :, t].unsqueeze(2).broadcast_to([P, 64, KD])
                    eng = nc.vector
                    if t == 0:
                        eng.tensor_tensor(o3, v3, ab, OP.mult)
                    else:
                        tm = tmpp.tile([P, 8 * KH], bf16, name="wtmp")
                        tm3 = tm[:].rearrange("p (g d) -> p g d", d=KD)
                        eng.tensor_tensor(tm3, v3, ab, OP.mult)
                        nc.vector.tensor_add(o[:], o[:], tm[:])

                # ---- o -> bf16 -> DRAM -> xbar transpose -> opad ----
                if stages < 5: continue
                nc.scalar.copy(obf[:], o[:])
                odr = dramp.tile([8 * P, KH], bf16, name="odr")
                nc.sync.dma_start(
                    odr[:].rearrange("(b r) h -> r b h", r=P),
                    obf[:].rearrange("p (b h) -> p b h", h=KH))
                for ht in range(4):
                    obt = tmpp.tile([P, HW], bf16, name="obt", tag="obt")
                    if no_xpose:
                        nc.sync.dma_start(obt[:].rearrange('p (a b) -> p a b', b=KH), odr[0:P * 2].rearrange('(p a) h -> p a h', p=P))
                    else:
                        nc.sync.dma_start_transpose(obt[:], odr[:, ht * P:(ht + 1) * P])
                    opv = opad[ht][:, 0:PHW].rearrange("c (i j) -> c i j", j=PW)
                    nc.sync.dma_start(
                        opv[:, 1:H + 1, 1:W + 1],
                        obt[:].rearrange("c (i j) -> c i j", j=W))

                # ---- conv3x3 #1 (bf16): y1 = W1 * opad ----
                if stages < 6: continue
                CHUNKS = [(0, 15), (15, 15), (30, 2)]
                for co in range(2):
                    for (i0, nr) in CHUNKS:
                        ps = convps.tile([P, 512], f32, name="c1ps", tag="cps")
                        nw = PW * nr
                        for tap in range(9):
                            ty, tx = tap // 3, tap % 3
                            for ci in range(4):
                                wt = wcp.tile([P, P], bf16, name="w1t")
                                nc.sync.dma_start(wt[:], w1[l, tap, ci, co])
                                base = PW * (i0 + ty) + tx
                                nc.tensor.matmul(
                                    ps[:, 0:nw], wt[:], opad[ci][:, base:base + nw],
                                    start=(tap == 0 and ci == 0),
                                    stop=(tap == 8 and ci == 3))
                        nc.scalar.copy(
                            y1[co][:, W * i0:W * (i0 + nr)].rearrange(
                                "c (i j) -> c i j", j=W),
                            ps[:, 0:nw].rearrange("c (i j) -> c i j", j=PW)[:, :, 0:W])

                # ---- BN stats + AllGather ----
                if stages < 7: continue
                for co in range(2):
                    nc.vector.tensor_reduce(out=st[:, 2 * co:2 * co + 1], in_=y1[co][:],
                                            axis=AX.X, op=OP.add)
                    nc.scalar.square(o[:, 0:HW], y1[co][:])
                    nc.vector.tensor_reduce(out=st[:, 2 * co + 1:2 * co + 2],
                                            in_=o[:, 0:HW], axis=AX.X, op=OP.add)
                if no_cc:
                    nc.vector.tensor_scalar_mul(gsum[:], st[:], float(ncores))
                else:
                    cci = dramp.tile([1, 512], f32, name="cci")
                    cco = dramp.tile([ncores, 512], f32, name="cco", addr_space="Shared")
                    nc.sync.dma_start(cci[0].rearrange("(p j) -> p j", j=4), st[:])
                    nc.gpsimd.collective_compute(
                        "AllGather", OP.bypass,
                        replica_groups=[list(range(ncores))],
                        ins=[cci.opt()], outs=[cco.opt()])
                    nc.sync.dma_start(
                        gst[:, 0:4 * ncores].rearrange("p (j s) -> p j s", s=ncores),
                        cco[:].rearrange("s (p j) -> p j s", j=4))
                    nc.vector.tensor_reduce(
                        out=gsum[:], in_=gst[:, 0:4 * ncores].rearrange("p (j s) -> p j s", s=ncores),
                        axis=AX.X, op=OP.add)

                # ---- BN coefficients: A = g/sqrt(var+eps), B = b - mean*A ----
                if stages < 8: continue
                NTOT = float(ncores * HW)
                for co in range(2):
                    nc.vector.tensor_scalar_mul(t1[co][:], gsum[:, 2 * co:2 * co + 1],
                                                1.0 / NTOT)
                    nc.vector.tensor_scalar_mul(vart[:], gsum[:, 2 * co + 1:2 * co + 2],
                                                1.0 / NTOT)
                    nc.vector.tensor_mul(sq[:], t1[co][:], t1[co][:])
                    nc.vector.tensor_sub(vart[:], vart[:], sq[:])
                    nc.vector.tensor_scalar_add(vart[:], vart[:], BN_EPS)
                    nc.scalar.activation(stdt[:], vart[:], ACTF.Sqrt)
                    nc.vector.reciprocal(stdt[:], stdt[:])
                    nc.vector.tensor_mul(Ac[co][:], bngt[co][:], stdt[:])
                    nc.vector.tensor_mul(sq[:], t1[co][:], Ac[co][:])
                    nc.vector.tensor_sub(Bc[co][:], bnbt[co][:], sq[:])
                    # h1 = relu(A*y1 + B), strided bf16 into padded conv2 input
                    h1v = h1p[co][:, 0:PHW].rearrange("c (i j) -> c i j", j=PW)
                    nc.scalar.activation(
                        h1v[:, 1:H + 1, 1:W + 1],
                        y1[co][:].rearrange("c (i j) -> c i j", j=W),
                        ACTF.Relu, bias=Bc[co][:], scale=Ac[co][:])

                # ---- conv3x3 #2 (bf16) + residual update ----
                if stages < 9: continue
                for co in range(2):
                    nc.scalar.add(x[co][:], x[co][:], gob2t[co][:])
                    for (i0, nr) in CHUNKS:
                        ps = convps.tile([P, 512], f32, name="c2ps", tag="cps")
                        nw = PW * nr
                        for tap in range(9):
                            ty, tx = tap // 3, tap % 3
                            for ci in range(2):
                                wt = wcp.tile([P, P], bf16, name="w1t")
                                nc.sync.dma_start(wt[:], w2[l, tap, ci, co])
                                base = PW * (i0 + ty) + tx
                                nc.tensor.matmul(
                                    ps[:, 0:nw], wt[:], h1p[ci][:, base:base + nw],
                                    start=(tap == 0 and ci == 0),
                                    stop=(tap == 8 and ci == 1))
                        xslice = x[co][:, W * i0:W * (i0 + nr)]
                        nc.vector.scalar_tensor_tensor(
                            out=xslice.rearrange("c (i j) -> c i j", j=W),
                            in0=ps[:, 0:nw].rearrange("c (i j) -> c i j", j=PW)[:, :, 0:W],
                            scalar=gamt[:],
                            in1=xslice.rearrange("c (i j) -> c i j", j=W),
                            op0=OP.mult, op1=OP.add)
                    if l < layers - 1:
                        nc.scalar.copy(xb[co][:], x[co][:])
                    else:
                        nc.sync.dma_start(out[co * P:(co + 1) * P, :], x[co][:])
                if dbg and l == DBGL:
                    fcvt = mp.tile([P, 8 * KH], f32, name="fcvt")
                    for nm, src_t in [("d_q", qbt), ("d_k", kbt[l]), ("d_v", vbt[l]),
                                      ("d_o", o)]:
                        nc.vector.tensor_copy(fcvt[:], src_t[:])
                        nc.sync.dma_start(dbgt[nm], fcvt[:])
                    nc.vector.tensor_copy(fcvt[:, 0:320], S[:])
                    nc.sync.dma_start(dbgt["d_S"], fcvt[:, 0:320])
                    nc.vector.tensor_copy(fcvt[:, 0:320], attn[:])
                    nc.sync.dma_start(dbgt["d_attn"], fcvt[:, 0:320])
                    nc.vector.tensor_copy(fcvt[:, 0:PHW], opad[0][:, 0:PHW])
                    nc.sync.dma_start(dbgt["d_opad0"], fcvt[:, 0:PHW])
                    nc.sync.dma_start(dbgt["d_y1_0"], y1[0][:])
                    nc.sync.dma_start(dbgt["d_gsum"], gsum[:])
                    nc.sync.dma_start(dbgt["d_A0"], Ac[0][:])
                    nc.sync.dma_start(dbgt["d_B0"], Bc[0][:])
                    nc.vector.tensor_copy(fcvt[:, 0:PHW], h1p[0][:, 0:PHW])
                    nc.sync.dma_start(dbgt["d_h1p0"], fcvt[:, 0:PHW])
                    nc.sync.dma_start(dbgt["d_x0"], x[0][:])

    nc.compile()
    return nc


def _host_prep(inputs):
    bf = ml_dtypes.bfloat16
    kw, kb, qw, qb = inputs["kw"], inputs["kb"], inputs["qw"], inputs["qb"]
    vw, vb = inputs["vw"], inputs["vb"]
    ow1, ow2 = inputs["ow1"], inputs["ow2"]
    gammas, ob2 = inputs["gammas"], inputs["ob2"]

    def packw(wm):  # [L, KH, C] -> [L, 2, 128, KH]
        return np.ascontiguousarray(
            wm.transpose(0, 2, 1).reshape(L, 2, P, KH)).astype(bf)

    d = {}
    d["wq"] = packw(qw / 8.0)
    d["wk"] = packw(kw)
    d["wv"] = packw(vw)
    d["bq"] = np.ascontiguousarray((qb / 8.0).reshape(L, 1, KH)).astype(bf)
    d["bk"] = np.ascontiguousarray(kb.reshape(L, 1, KH)).astype(bf)
    d["bv"] = np.ascontiguousarray(vb.reshape(L, 1, KH)).astype(bf)
    # ow1 [L, 256, 512, 3, 3] -> [L, tap, ci(4), co(2), a(cin128), b(cout128)]
    a1 = ow1.reshape(L, 2, P, 4, P, 3, 3).transpose(0, 5, 6, 3, 1, 4, 2)
    d["w1"] = np.ascontiguousarray(a1.reshape(L, 9, 4, 2, P, P)).astype(bf)
    a2 = ow2.reshape(L, 2, P, 2, P, 3, 3).transpose(0, 5, 6, 3, 1, 4, 2)
    d["w2"] = np.ascontiguousarray(a2.reshape(L, 9, 2, 2, P, P)).astype(bf)
    d["bngd"] = np.ascontiguousarray(
        inputs["bn_g"].reshape(L, 2, P, 1)).astype(np.float32)
    d["bnbd"] = np.ascontiguousarray(
        inputs["bn_b"].reshape(L, 2, P, 1)).astype(np.float32)
    gob2 = gammas[:, None] * ob2
    d["gob2d"] = np.ascontiguousarray(gob2.reshape(L, 2, P, 1)).astype(np.float32)
    d["gamd"] = np.ascontiguousarray(
        np.broadcast_to(gammas[:, None, None], (L, P, 1))).astype(np.float32)
    return d


def kernel(**inputs):
    if "nc" not in _compiled:
        _compiled["nc"] = _build()
    nc = _compiled["nc"]
    shared = _host_prep(inputs)
    x = np.ascontiguousarray(inputs["x"].reshape(B, C, HW)).astype(np.float32)
    in_maps = []
    for c in range(NC):
        m = dict(shared)
        m["xin"] = x[c]
        in_maps.append(m)
    res = bass_utils.run_bass_kernel_spmd(nc, in_maps, core_ids=list(range(NC)))
    outs = np.stack([res.results[c]["out"] for c in range(NC)])
    return outs.reshape(B, C, H, W).astype(np.float32)



# revision 9
# speedup vs baseline: 2.2873x; 2.2873x over previous
"""AttentiveDensenet Trainium2 Bass kernel (v2).

Data-parallel over batch B=8 across 8 NeuronCores (1 image per core).

v2 changes vs v1 (driven by trace analysis of the 1.52ms baseline):
  - Conv weights are host-packed so each layer's w1/w2 load as ONE big
    DMA each into resident SBUF tiles (double-buffered, prefetched a
    layer ahead). Kills the per-tile weight DMA storm (1392 SP DMA
    issues, 299k tiny packets) that starved the PE and kept the HAM
    clock gate cold (PE ran at 1.2 GHz for 87% of the run).
  - o is transposed to channel-major with PE transpose-mode matmuls
    (32x [128,128] blocks/layer) + ACT evac into the padded conv input,
    replacing the DRAM bounce + xbar-transpose chain (~40us/layer of
    dead time).
  - Attention is emitted per quarter-image (2 pos-blocks) and conv1 in
    8-row chunks interleaved with the o-transposes, so DVE attention
    overlaps PE conv work instead of serializing.
  - Weighted sum accumulates bf16 products with a pair tree (2x DVE
    mode) instead of sequential f32 adds.
  - A dummy AllGather at kernel start absorbs the ~47us core-launch
    skew barrier under layer-0 compute; per-layer BN stat AllGathers
    then run near their intrinsic latency.
  - BN stats are reduced per conv1-chunk as results land (DVE is idle
    then), so the AllGather starts immediately after the last chunk.
"""
import numpy as np
import ml_dtypes

import concourse.bacc as bacc
import concourse.mybir as mybir
import concourse.tile as tile
from concourse import bass_utils

L, C, B, H, W = 4, 256, 8, 32, 32
NH, KD = 8, 64
KH = NH * KD          # 512
HW = H * W            # 1024
P = 128
NC = 8                # cores
TOPK = 4
EPS = 1e-7
BN_EPS = 1e-5
PW = W + 2            # 34
PHW = PW * (H + 2)    # 1156
NQ = 4                # quarters (2 pos-blocks each)
RPC = 8               # conv chunk rows
NCH = H // RPC        # 4 conv chunks
CW = PW * RPC         # 272 conv chunk width (incl pad cols)

f32 = mybir.dt.float32
bf16 = mybir.dt.bfloat16
AX = mybir.AxisListType
OP = mybir.AluOpType
ACTF = mybir.ActivationFunctionType

_compiled = {}


def _build(ncores=NC, layers=L, stages=99):
    nc = bacc.Bacc(None, target_bir_lowering=False, debug=False, num_devices=ncores)

    # ---- DRAM I/O (per-core shapes; weights replicated) ----
    xin = nc.dram_tensor("xin", [C, HW], f32, kind="ExternalInput").ap()
    wq = nc.dram_tensor("wq", [L, P, 2 * KH], bf16, kind="ExternalInput").ap()
    wk = nc.dram_tensor("wk", [L, P, 2 * KH], bf16, kind="ExternalInput").ap()
    wv = nc.dram_tensor("wv", [L, P, 2 * KH], bf16, kind="ExternalInput").ap()
    bq = nc.dram_tensor("bq", [L, 1, KH], bf16, kind="ExternalInput").ap()
    bk = nc.dram_tensor("bk", [L, 1, KH], bf16, kind="ExternalInput").ap()
    bv = nc.dram_tensor("bv", [L, 1, KH], bf16, kind="ExternalInput").ap()
    w1d = nc.dram_tensor("w1d", [L, P, 72 * P], bf16, kind="ExternalInput").ap()
    w2d = nc.dram_tensor("w2d", [L, P, 36 * P], bf16, kind="ExternalInput").ap()
    bngd = nc.dram_tensor("bngd", [L, 2, P, 1], f32, kind="ExternalInput").ap()
    bnbd = nc.dram_tensor("bnbd", [L, 2, P, 1], f32, kind="ExternalInput").ap()
    gob2d = nc.dram_tensor("gob2d", [L, 2, P, 1], f32, kind="ExternalInput").ap()
    gamd = nc.dram_tensor("gamd", [L, P, 1], f32, kind="ExternalInput").ap()
    identd = nc.dram_tensor("identd", [P, P], bf16, kind="ExternalInput").ap()
    out = nc.dram_tensor("out", [C, HW], f32, kind="ExternalOutput").ap()

    with tile.TileContext(nc) as tc:
        with tc.tile_pool(name="main", bufs=1) as mp, \
             tc.tile_pool(name="prodp", bufs=5) as prodp, \
             tc.tile_pool(name="sprod", bufs=3) as sprod, \
             tc.tile_pool(name="wkvp", bufs=6) as wkvp, \
             tc.tile_pool(name="biasp", bufs=6) as biasp, \
             tc.tile_pool(name="kqvps", bufs=2, space="PSUM") as kqvps, \
             tc.tile_pool(name="convps", bufs=4, space="PSUM") as convps, \
             tc.tile_pool(name="xps", bufs=2, space="PSUM") as xps, \
             tc.tile_pool(name="dramp", bufs=2, space="DRAM") as dramp:

            # ---- persistent tiles ----
            x = [mp.tile([P, HW], f32, name=f"x{i}") for i in range(2)]
            xb = [mp.tile([P, HW], bf16, name=f"xb{i}") for i in range(2)]
            qbt = mp.tile([P, 8 * KH], bf16, name="qbt")
            kbt = [mp.tile([P, 8 * KH], bf16, name=f"kbt{i}") for i in range(L)]
            vbt = [mp.tile([P, 8 * KH], bf16, name=f"vbt{i}") for i in range(L)]
            S = mp.tile([P, 64 * 5], f32, name="S")
            attn = mp.tile([P, 64 * 5], f32, name="attn")
            attnb = mp.tile([P, 64 * 5], bf16, name="attnb")
            mx = mp.tile([P, 64], f32, name="mx")
            zs = mp.tile([P, 64], f32, name="zs")
            dmin = mp.tile([P, 64], f32, name="dmin")
            mxp = mp.tile([P, 64], f32, name="mxp")
            o = mp.tile([P, 8 * KH], bf16, name="o")
            opad = [mp.tile([P, PHW + 2], bf16, name=f"opad{i}") for i in range(4)]
            y1 = [mp.tile([P, HW], f32, name=f"y1_{i}") for i in range(2)]
            h1p = [mp.tile([P, PHW + 2], bf16, name=f"h1p{i}") for i in range(2)]
            st = mp.tile([P, 16], f32, name="st")       # (co, kind, chunk)
            st2 = mp.tile([P, 4], f32, name="st2")      # (co, kind)
            gst = mp.tile([P, 32], f32, name="gst")
            gsum = mp.tile([P, 4], f32, name="gsum")
            ones1 = mp.tile([1, P], bf16, name="ones1")
            ident = mp.tile([P, P], bf16, name="ident")
            wres = mp.tile([1, 8], f32, name="wres")
            wsrc = mp.tile([1, 8], f32, name="wsrc")
            # resident conv weights, double-buffered across layers
            w1t = [mp.tile([P, 72 * P], bf16, name=f"w1t{i}") for i in range(2)]
            w2t = [mp.tile([P, 36 * P], bf16, name=f"w2t{i}") for i in range(2)]
            # per-layer consts
            bngt = [mp.tile([P, 1], f32, name=f"bngt{i}") for i in range(2)]
            bnbt = [mp.tile([P, 1], f32, name=f"bnbt{i}") for i in range(2)]
            gob2t = [mp.tile([P, 1], f32, name=f"gob2t{i}") for i in range(2)]
            gamt = mp.tile([P, 1], f32, name="gamt")
            # BN scratch
            t1 = [mp.tile([P, 1], f32, name=f"t1_{i}") for i in range(2)]
            Ac = [mp.tile([P, 1], f32, name=f"Ac{i}") for i in range(2)]
            Bc = [mp.tile([P, 1], f32, name=f"Bc{i}") for i in range(2)]
            sq1 = mp.tile([P, 1], f32, name="sq1")
            vart = mp.tile([P, 1], f32, name="vart")
            stdt = mp.tile([P, 1], f32, name="stdt")

            # ---- init ----
            for i in range(2):
                nc.sync.dma_start(x[i][:], xin[i * P:(i + 1) * P, :])
                nc.scalar.copy(xb[i][:], x[i][:])
            nc.sync.dma_start(ident[:], identd)
            for i in range(4):
                nc.vector.memset(opad[i][:], 0)
            for i in range(2):
                nc.vector.memset(h1p[i][:], 0)
            nc.vector.memset(ones1[:], 1.0)
            nc.vector.memset(S[:], 0)
            nc.vector.memset(attn[:], 0)
            nc.vector.memset(wsrc[:], 0)

            # warmup collective: absorbs core-launch skew + CC init
            # barrier while layer-0 compute runs. wres (all zeros) is
            # consumed additively just before the output DMA so DCE
            # can't drop the chain.
            wrmi = dramp.tile([1, 8], f32, name="wrmi")
            wrmo = dramp.tile([ncores, 8], f32, name="wrmo", addr_space="Shared")
            nc.sync.dma_start(wrmi[:], wsrc[:])
            nc.gpsimd.collective_compute(
                "AllGather", OP.bypass,
                replica_groups=[list(range(ncores))],
                ins=[wrmi.opt()], outs=[wrmo.opt()])
            nc.sync.dma_start(wres[:], wrmo[0:1, :])

            # layer-0 conv weights
            nc.sync.dma_start(w1t[0][:], w1d[0])
            nc.sync.dma_start(w2t[0][:], w2d[0])

            S3 = S[:].rearrange("p (g t) -> p g t", t=5)
            at3 = attn[:].rearrange("p (g t) -> p g t", t=5)
            ab3 = attnb[:].rearrange("p (g t) -> p g t", t=5)


            for l in range(layers):
                R = l + 1      # number of real keys
                T = R + 1      # +1 zero key

                # ---- per-layer consts ----
                for i in range(2):
                    nc.sync.dma_start(bngt[i][:], bngd[l, i])
                    nc.sync.dma_start(bnbt[i][:], bnbd[l, i])
                    nc.sync.dma_start(gob2t[i][:], gob2d[l, i])
                nc.sync.dma_start(gamt[:], gamd[l])

                # ---- K/Q/V 1x1 convs, position-major ----
                wts, bts = {}, {}
                for name, wdr, bdr in (("k", wk, bk), ("v", wv, bv), ("q", wq, bq)):
                    bt = biasp.tile([1, KH], bf16, name=f"bias_{name}", tag="bias")
                    nc.sync.dma_start(bt[:], bdr[l])
                    wt = wkvp.tile([P, 2 * KH], bf16, name=f"w_{name}", tag="wkv")
                    nc.sync.dma_start(wt[:], wdr[l])
                    wts[name], bts[name] = wt, bt
                dests = {"k": kbt[l][:], "v": vbt[l][:], "q": qbt[:]}
                for pb in range(8):
                    for name in ("k", "v", "q"):
                        ps = kqvps.tile([P, KH], f32, name="kqv_ps")
                        nc.tensor.matmul(ps[:], ones1[:], bts[name][:],
                                         start=True, stop=False)
                        nc.tensor.matmul(ps[:], xb[0][:, pb * P:(pb + 1) * P],
                                         wts[name][:, 0:KH], start=False, stop=False)
                        nc.tensor.matmul(ps[:], xb[1][:, pb * P:(pb + 1) * P],
                                         wts[name][:, KH:2 * KH], start=False, stop=True)
                        nc.scalar.copy(dests[name][:, pb * KH:(pb + 1) * KH], ps[:])

                if stages < 2:
                    continue

                # ---- attention, per quarter (2 pos-blocks) ----
                for qt in range(NQ):
                    g0 = 16 * qt
                    fs = 1024 * qt           # free-dim start in [128, 4096] tiles
                    qsl = slice(fs, fs + 1024)
                    # scores
                    for t in range(R):
                        pr = sprod.tile([P, 1024], bf16, name="sprodt", tag="sp")
                        nc.vector.tensor_mul(pr[:], qbt[:, qsl], kbt[t][:, qsl])
                        nc.vector.tensor_reduce(
                            out=S3[:, g0:g0 + 16, t],
                            in_=pr[:].rearrange("p (g d) -> p g d", d=KD),
                            axis=AX.X, op=OP.add)
                    nc.vector.memset(S3[:, g0:g0 + 16, R:R + 1], 0)  # zero key

                    if stages < 3:
                        continue
                    # softmax over T slots
                    mxq = mx[:, g0:g0 + 16]
                    zsq = zs[:, g0:g0 + 16]
                    nc.vector.tensor_reduce(out=mxq, in_=S3[:, g0:g0 + 16, 0:T],
                                            axis=AX.X, op=OP.max)
                    nc.vector.tensor_tensor(
                        at3[:, g0:g0 + 16, 0:T], S3[:, g0:g0 + 16, 0:T],
                        mxq.unsqueeze(2).broadcast_to([P, 16, T]), OP.subtract)
                    nc.scalar.activation(at3[:, g0:g0 + 16, 0:T],
                                         at3[:, g0:g0 + 16, 0:T], ACTF.Exp)
                    nc.vector.tensor_reduce(out=zsq, in_=at3[:, g0:g0 + 16, 0:T],
                                            axis=AX.X, op=OP.add)
                    nc.vector.reciprocal(zsq, zsq)
                    nc.vector.tensor_tensor(
                        at3[:, g0:g0 + 16, 0:T], at3[:, g0:g0 + 16, 0:T],
                        zsq.unsqueeze(2).broadcast_to([P, 16, T]), OP.mult)

                    # sparse top-k (only T=5): exact 4th-largest via
                    # 2nd-smallest = min of pairwise maxes
                    if T > TOPK:
                        dmq = dmin[:, g0:g0 + 16]
                        mpq = mxp[:, g0:g0 + 16]
                        first = True
                        for i in range(T):
                            for j in range(i + 1, T):
                                dst = dmq if first else mpq
                                nc.vector.tensor_tensor(
                                    dst, at3[:, g0:g0 + 16, i],
                                    at3[:, g0:g0 + 16, j], OP.max)
                                if not first:
                                    nc.vector.tensor_tensor(dmq, dmq, mpq, OP.min)
                                first = False
                        nc.vector.tensor_scalar_add(dmq, dmq, EPS)
                        nc.vector.tensor_tensor(
                            at3[:, g0:g0 + 16, 0:T], at3[:, g0:g0 + 16, 0:T],
                            dmq.unsqueeze(2).broadcast_to([P, 16, T]), OP.subtract)
                        nc.vector.tensor_scalar_max(at3[:, g0:g0 + 16, 0:T],
                                                    at3[:, g0:g0 + 16, 0:T], 0.0)
                        nc.vector.tensor_reduce(out=zsq, in_=at3[:, g0:g0 + 16, 0:T],
                                                axis=AX.X, op=OP.add)
                        nc.vector.tensor_scalar_add(zsq, zsq, EPS)
                        nc.vector.reciprocal(zsq, zsq)
                        nc.vector.tensor_tensor(
                            at3[:, g0:g0 + 16, 0:T], at3[:, g0:g0 + 16, 0:T],
                            zsq.unsqueeze(2).broadcast_to([P, 16, T]), OP.mult)

                    nc.vector.tensor_copy(attnb[:, 80 * qt:80 * qt + 80],
                                          attn[:, 80 * qt:80 * qt + 80])

                    if stages < 4:
                        continue
                    # weighted sum: o_q = sum_t attn_t * v_t  (bf16 pair tree)
                    oq = o[:, qsl].rearrange("p (g d) -> p g d", d=KD)

                    def wprod(dst3, t):
                        nc.vector.tensor_tensor(
                            dst3, vbt[t][:, qsl].rearrange("p (g d) -> p g d", d=KD),
                            ab3[:, g0:g0 + 16, t].unsqueeze(2).broadcast_to(
                                [P, 16, KD]), OP.mult)

                    if R == 1:
                        wprod(oq, 0)
                    else:
                        pts = []
                        for t in range(R):
                            pt = prodp.tile([P, 1024], bf16, name="wprod", tag="wp")
                            wprod(pt[:].rearrange("p (g d) -> p g d", d=KD), t)
                            pts.append(pt)
                        if R == 2:
                            nc.vector.tensor_add(o[:, qsl], pts[0][:], pts[1][:])
                        elif R == 3:
                            t01 = prodp.tile([P, 1024], bf16, name="wprod", tag="wp")
                            nc.vector.tensor_add(t01[:], pts[0][:], pts[1][:])
                            nc.vector.tensor_add(o[:, qsl], t01[:], pts[2][:])
                        else:
                            t01 = prodp.tile([P, 1024], bf16, name="wprod", tag="wp")
                            t23 = prodp.tile([P, 1024], bf16, name="wprod", tag="wp")
                            nc.vector.tensor_add(t01[:], pts[0][:], pts[1][:])
                            nc.vector.tensor_add(t23[:], pts[2][:], pts[3][:])
                            nc.vector.tensor_add(o[:, qsl], t01[:], t23[:])

                    if stages < 5:
                        continue
                    # PE transpose o -> channel-major opad for this quarter
                    for pb in (2 * qt, 2 * qt + 1):
                        for ht in range(4):
                            tps = xps.tile([P, 1024], bf16, name="xpose_ps")
                            nc.tensor.transpose(
                                tps[:, 0:P],
                                o[:, pb * KH + ht * P:pb * KH + (ht + 1) * P],
                                ident[:])
                            opv = opad[ht][:, 0:PHW].rearrange(
                                "c (i j) -> c i j", j=PW)
                            nc.scalar.copy(
                                opv[:, 1 + 4 * pb:5 + 4 * pb, 1:W + 1],
                                tps[:, 0:P].rearrange("c (i j) -> c i j", j=W))

                if stages < 6:
                    continue
                # ---- conv3x3 #1: y1 = W1 * opad, 8-row chunks ----
                wl1 = w1t[l % 2]
                for c in range(NCH):
                    i0 = c * RPC
                    for co in range(2):
                        ps = convps.tile([P, 512], f32, name="c1ps", tag="cps")
                        for tap in range(9):
                            ty, tx = tap // 3, tap % 3
                            for ci in range(4):
                                f = (tap * 4 + ci) * 2 + co
                                base = PW * (i0 + ty) + tx
                                nc.tensor.matmul(
                                    ps[:, 0:CW], wl1[:, f * P:(f + 1) * P],
                                    opad[ci][:, base:base + CW],
                                    start=(tap == 0 and ci == 0),
                                    stop=(tap == 8 and ci == 3))
                        nc.scalar.copy(
                            y1[co][:, W * i0:W * (i0 + RPC)].rearrange(
                                "c (i j) -> c i j", j=W),
                            ps[:, 0:CW].rearrange("c (i j) -> c i j", j=PW)[:, :, 0:W])
                        # per-chunk BN stats while DVE is otherwise idle
                        # st columns: (co*2 + kind)*4 + chunk
                        ysl = y1[co][:, W * i0:W * (i0 + RPC)]
                        i_sum = (co * 2 + 0) * 4 + c
                        i_sq = (co * 2 + 1) * 4 + c
                        nc.vector.tensor_reduce(out=st[:, i_sum:i_sum + 1], in_=ysl,
                                                axis=AX.X, op=OP.add)
                        sqt = sprod.tile([P, 1024], bf16, name="sqt", tag="sp")
                        nc.scalar.square(sqt[:, 0:W * RPC], ysl)
                        nc.vector.tensor_reduce(out=st[:, i_sq:i_sq + 1],
                                                in_=sqt[:, 0:W * RPC],
                                                axis=AX.X, op=OP.add)

                if stages < 7:
                    continue
                # ---- BN stats AllGather ----
                nc.vector.tensor_reduce(
                    out=st2[:], in_=st[:].rearrange("p (g c) -> p g c", c=4),
                    axis=AX.X, op=OP.add)
                cci = dramp.tile([1, 512], f32, name="cci")
                cco = dramp.tile([ncores, 512], f32, name="cco", addr_space="Shared")
                nc.sync.dma_start(cci[0].rearrange("(p j) -> p j", j=4), st2[:])
                nc.gpsimd.collective_compute(
                    "AllGather", OP.bypass,
                    replica_groups=[list(range(ncores))],
                    ins=[cci.opt()], outs=[cco.opt()])
                nc.sync.dma_start(
                    gst[:, 0:4 * ncores].rearrange("p (j s) -> p j s", s=ncores),
                    cco[:].rearrange("s (p j) -> p j s", j=4))
                nc.vector.tensor_reduce(
                    out=gsum[:],
                    in_=gst[:, 0:4 * ncores].rearrange("p (j s) -> p j s", s=ncores),
                    axis=AX.X, op=OP.add)

                # prefetch next layer's conv weights (overlaps conv2 + next KQV)
                if l + 1 < layers:
                    nc.sync.dma_start(w1t[(l + 1) % 2][:], w1d[l + 1])
                    nc.sync.dma_start(w2t[(l + 1) % 2][:], w2d[l + 1])

                if stages < 8:
                    continue
                # ---- BN coefficients: A = g/sqrt(var+eps), B = b - mean*A ----
                NTOT = float(ncores * HW)
                for co in range(2):
                    nc.vector.tensor_scalar_mul(t1[co][:], gsum[:, 2 * co:2 * co + 1],
                                                1.0 / NTOT)
                    nc.vector.tensor_scalar_mul(vart[:], gsum[:, 2 * co + 1:2 * co + 2],
                                                1.0 / NTOT)
                    nc.vector.tensor_mul(sq1[:], t1[co][:], t1[co][:])
                    nc.vector.tensor_sub(vart[:], vart[:], sq1[:])
                    nc.vector.tensor_scalar_add(vart[:], vart[:], BN_EPS)
                    nc.scalar.activation(stdt[:], vart[:], ACTF.Sqrt)
                    nc.vector.reciprocal(stdt[:], stdt[:])
                    nc.vector.tensor_mul(Ac[co][:], bngt[co][:], stdt[:])
                    nc.vector.tensor_mul(sq1[:], t1[co][:], Ac[co][:])
                    nc.vector.tensor_sub(Bc[co][:], bnbt[co][:], sq1[:])
                    # h1 = relu(A*y1 + B), strided bf16 into padded conv2 input
                    h1v = h1p[co][:, 0:PHW].rearrange("c (i j) -> c i j", j=PW)
                    nc.scalar.activation(
                        h1v[:, 1:H + 1, 1:W + 1],
                        y1[co][:].rearrange("c (i j) -> c i j", j=W),
                        ACTF.Relu, bias=Bc[co][:], scale=Ac[co][:])

                if stages < 9:
                    continue
                # ---- conv3x3 #2 + residual x += gamma*(h2 + ob2) ----
                wl2 = w2t[l % 2]
                for co in range(2):
                    nc.scalar.add(x[co][:], x[co][:], gob2t[co][:])
                for c in range(NCH):
                    i0 = c * RPC
                    for co in range(2):
                        ps = convps.tile([P, 512], f32, name="c2ps", tag="cps")
                        for tap in range(9):
                            ty, tx = tap // 3, tap % 3
                            for ci in range(2):
                                f = (tap * 2 + ci) * 2 + co
                                base = PW * (i0 + ty) + tx
                                nc.tensor.matmul(
                                    ps[:, 0:CW], wl2[:, f * P:(f + 1) * P],
                                    h1p[ci][:, base:base + CW],
                                    start=(tap == 0 and ci == 0),
                                    stop=(tap == 8 and ci == 1))
                        xslice = x[co][:, W * i0:W * (i0 + RPC)]
                        nc.vector.scalar_tensor_tensor(
                            out=xslice.rearrange("c (i j) -> c i j", j=W),
                            in0=ps[:, 0:CW].rearrange("c (i j) -> c i j", j=PW)[:, :, 0:W],
                            scalar=gamt[:],
                            in1=xslice.rearrange("c (i j) -> c i j", j=W),
                            op0=OP.mult, op1=OP.add)
                for co in range(2):
                    if l < layers - 1:
                        nc.scalar.copy(xb[co][:], x[co][:])
                    else:
                        if co == 0:
                            # consume the warmup-AllGather zeros (adds 0.0)
                            nc.vector.tensor_tensor(x[0][0:1, 0:8], x[0][0:1, 0:8],
                                                    wres[:], OP.add)
                        nc.sync.dma_start(out[co * P:(co + 1) * P, :], x[co][:])

    nc.compile()
    return nc


def _host_prep(inputs):
    bf = ml_dtypes.bfloat16
    kw, kb, qw, qb = inputs["kw"], inputs["kb"], inputs["qw"], inputs["qb"]
    vw, vb = inputs["vw"], inputs["vb"]
    ow1, ow2 = inputs["ow1"], inputs["ow2"]
    gammas, ob2 = inputs["gammas"], inputs["ob2"]

    def packw(wm):  # [L, KH, C] -> [L, 128, 2*KH]  (lhsT per ci-half, fused)
        a = wm.transpose(0, 2, 1).reshape(L, 2, P, KH)   # [L, ci, cin128, KH]
        return np.ascontiguousarray(a.transpose(0, 2, 1, 3).reshape(L, P, 2 * KH)
                                    ).astype(bf)

    d = {}
    d["wq"] = packw(qw / 8.0)
    d["wk"] = packw(kw)
    d["wv"] = packw(vw)
    d["bq"] = np.ascontiguousarray((qb / 8.0).reshape(L, 1, KH)).astype(bf)
    d["bk"] = np.ascontiguousarray(kb.reshape(L, 1, KH)).astype(bf)
    d["bv"] = np.ascontiguousarray(vb.reshape(L, 1, KH)).astype(bf)
    # ow1 [L, 256, 512, 3, 3] -> [L, cin128, f=(tap, ci4, co2), cout128]
    a1 = ow1.reshape(L, 2, P, 4, P, 3, 3)   # [L, co, cout, ci, cin, ty, tx]
    a1 = a1.transpose(0, 4, 5, 6, 3, 1, 2)  # [L, cin, ty, tx, ci, co, cout]
    d["w1d"] = np.ascontiguousarray(a1.reshape(L, P, 72 * P)).astype(bf)
    a2 = ow2.reshape(L, 2, P, 2, P, 3, 3)
    a2 = a2.transpose(0, 4, 5, 6, 3, 1, 2)  # [L, cin, ty, tx, ci, co, cout]
    d["w2d"] = np.ascontiguousarray(a2.reshape(L, P, 36 * P)).astype(bf)
    d["bngd"] = np.ascontiguousarray(
        inputs["bn_g"].reshape(L, 2, P, 1)).astype(np.float32)
    d["bnbd"] = np.ascontiguousarray(
        inputs["bn_b"].reshape(L, 2, P, 1)).astype(np.float32)
    gob2 = gammas[:, None] * ob2
    d["gob2d"] = np.ascontiguousarray(gob2.reshape(L, 2, P, 1)).astype(np.float32)
    d["gamd"] = np.ascontiguousarray(
        np.broadcast_to(gammas[:, None, None], (L, P, 1))).astype(np.float32)
    d["identd"] = np.eye(P, dtype=np.float32).astype(bf)
    return d


def kernel(**inputs):
    if "nc" not in _compiled:
        _compiled["nc"] = _build()
    nc = _compiled["nc"]
    shared = _host_prep(inputs)
    x = np.ascontiguousarray(inputs["x"].reshape(B, C, HW)).astype(np.float32)
    in_maps = []
    for c in range(NC):
        m = dict(shared)
        m["xin"] = x[c]
        in_maps.append(m)
    res = bass_utils.run_bass_kernel_spmd(nc, in_maps, core_ids=list(range(NC)))
    outs = np.stack([res.results[c]["out"] for c in range(NC)])
    return outs.reshape(B, C, H, W).astype(np.float32)
